# revision 30
# baseline (speedup 1.0000x reference)
"""Trainium2 Bass kernel for nn_Attention (B=16,N=4096,C=1024,H=16,HD=64,Q=64).

Data-parallel over B across 8 NeuronCores (2 batches/core). Per batch the
attention is reassociated so no k/v tensors are materialized and no on-chip
transposes are needed:

  q^T = Wq @ x_q^T                      [(h,d)=1024, 64]
  G_h^T = Wk_h^T @ q_h                  G^T: [c=1024, (h,q)=1024]
  S^T   = x @ G^T   (per t-tile)        [t, (h,q)]   (contract c)
  p^T   = exp(S^T / 8)                  (softmax w/o max-sub: scores ~ +-5)
  u^T   = x^T(nat) @ p^T  (accum t)     [c, (h,q)]   (contract t)
  den   = ones @ pacc     (pacc: GpSimd p-sum over t)
  o_h^T = (Wv_h^T)^T @ u_h^T, scaled by 1/den at PSUM eviction
  y     = o^T.T @ Wproj^T + b           [64, 1024]   (contract (h,d))

Optimizations over the first working version:
  - startup: consolidated/prioritized DMAs (order xq2 -> wq -> x-block0 ->
    wk -> xnt0; wv/wp deferred past block 2) with per-ck granularity so the
    q/G matmuls pipeline with the weight stream; wk pre-tiled on host so its
    DMA runs full-bandwidth 2KB lines.
  - q matmuls packed across both batches (N=128), G matmuls packed (N=256),
    PSUM evictions split across DVE and ACT.
  - batch 1's block 0 is emitted before batch 0's epilogue so the PE never
    drains at the batch boundary (keeps HAM at K=8/8).
  - softmax denominators: bf16 pacc copy -> all-ones matmul replicates den
    across partitions (no GpSimd broadcast), reciprocal_approx_fast, rdo via
    two strided ACT copies; dn PSUM tiles live in the u pool so o matmuls
    keep all 4 psa banks.
  - last batch's tail: den chain + o matmuls interleaved into the final u
    rounds (o first-half runs while the last u round computes), so only the
    last 4 PSUM evictions + o second-half + proj remain after the last u
    matmul.
"""
import os
import numpy as np

B, N, C = 16, 4096, 1024
H, HD, QL = 16, 64, 64
BL = B // 8           # batches per core
CK = C // 128         # 8 c-tiles
TB = 512              # tokens per t-block
NBLK = N // TB        # 8 blocks
TPB = TB // 128       # 4 t-tiles per block
HQ = H * QL           # 1024
SCALE = HD ** -0.5

_CACHE = {}


def _build():
    import concourse.bass as bass
    import concourse.tile as tile
    from concourse import bacc, mybir

    f32 = mybir.dt.float32
    bf16 = mybir.dt.bfloat16
    EXP = mybir.ActivationFunctionType.Exp
    CPY = mybir.ActivationFunctionType.Copy

    nc = bacc.Bacc("TRN2", target_bir_lowering=False, debug=False, num_devices=8)
    xn = nc.dram_tensor("xn", [BL, N, C], bf16, kind="ExternalInput").ap()
    xt = nc.dram_tensor("xt", [BL, C, N], bf16, kind="ExternalInput").ap()
    wq = nc.dram_tensor("wq", [C, C], bf16, kind="ExternalInput").ap()   # Wq^T
    # Wk pre-tiled on host: wk2[p, ck*1024+pair*128+m] = Wk[pair*128+p, ck*128+m]
    wk2 = nc.dram_tensor("wk2", [128, CK * 1024], bf16, kind="ExternalInput").ap()
    wv = nc.dram_tensor("wv", [C, C], bf16, kind="ExternalInput").ap()   # Wv^T
    wp = nc.dram_tensor("wp", [C, C], bf16, kind="ExternalInput").ap()   # Wproj^T
    bp = nc.dram_tensor("bp", [1, C], bf16, kind="ExternalInput").ap()
    # xq2[c, b*64+q] = x[b, q, c] for this core's two batches
    xq2 = nc.dram_tensor("xq2", [C, 2 * QL], bf16, kind="ExternalInput").ap()
    y = nc.dram_tensor("y", [BL, QL, C], f32, kind="ExternalOutput").ap()

    with tile.TileContext(nc) as tc:
        with (
            tc.tile_pool(name="wpool", bufs=2) as wpool,
            tc.tile_pool(name="xpool", bufs=2) as xpool,
            tc.tile_pool(name="gpool", bufs=1) as gpool,
            tc.tile_pool(name="upool", bufs=1) as upool,
            tc.tile_pool(name="small", bufs=1) as small,
            tc.tile_pool(name="ptp", bufs=3) as ptp,
            tc.tile_pool(name="psa", bufs=4, space="PSUM") as psa,
            tc.tile_pool(name="psu", bufs=4, space="PSUM") as psu,
        ):
            # ---------- memsets first: no DMA deps, run under the launch ----
            ones128 = small.tile([128, 128], bf16, tag="ones128")
            nc.gpsimd.memset(ones128[:], 1.0)
            qbd2 = small.tile([128, 8 * 256], bf16, tag="qbd2")
            nc.gpsimd.memset(qbd2[:], 0.0)
            paccs = []
            for b in range(BL):
                pacc = small.tile([128, HQ], f32, tag="pacc", bufs=2,
                                  name=f"pacc{b}")
                nc.gpsimd.memset(pacc[:], 0.0)
                paccs.append(pacc)

            # ---------- startup DMAs in dependency-priority order ----------
            xqt2 = small.tile([128, CK * 128], bf16, tag="xqt")
            nc.sync.dma_start(
                xqt2[:].rearrange("p (t q) -> p t q", t=CK),
                xq2[:, :].rearrange("(t p) q -> p t q", p=128))
            # Wq^T per c-tile so q matmuls pipeline with the stream
            wt = wpool.tile([128, 8 * 1024], bf16, tag="w", name="wt_q")
            for ck in range(CK):
                nc.sync.dma_start(wt[:, ck * 1024:(ck + 1) * 1024],
                                  wq[ck * 128:(ck + 1) * 128, :])
            # block-0 x^T ahead of Wk: S starts as soon as G's first
            # c-tiles are evicted, overlapping the tail of the wk stream
            xtt0 = xpool.tile([128, CK * TB], bf16, tag="xt", bufs=2)
            nc.sync.dma_start(
                xtt0[:].rearrange("p (t n) -> p t n", t=CK),
                xt[0, :, 0:TB].rearrange("(t p) n -> p t n", p=128))
            # Wk host-pre-tiled column-tile-major: wt2[p, ck*1024+pair*128+m]
            #   = wk[pair*128 + p, ck*128 + m]; G's ck-tiles unblock in order.
            # (vector-engine queue: streams in parallel with wq on sync)
            wt2 = wpool.tile([128, 8 * 1024], bf16, tag="w", name="wt_k")
            for ck in range(CK):
                nc.sync.dma_start(wt2[:, ck * 1024:(ck + 1) * 1024],
                                    wk2[:, ck * 1024:(ck + 1) * 1024])
            xnt0 = xpool.tile([128, TPB * 1024], bf16, tag="xn", bufs=3)
            nc.sync.dma_start(
                xnt0[:].rearrange("p (t c) -> p t c", t=TPB),
                xn[0, 0:TB, :].rearrange("(t p) c -> p t c", p=128))
            bps = small.tile([128, C], bf16, tag="bps")
            nc.sync.dma_start(bps[0:1, :], bp[:, :])
            bpf = small.tile([128, C], bf16, tag="bpf")
            nc.gpsimd.partition_broadcast(bpf[:], bps[0:1, :])

            # ---------- PE warm-up: ~12 junk matmuls under the DMA wait ----
            # (HAM releases the 4/8 clock throttle after ~3.4us of sustained
            # PE activity; this burst runs while wq streams so the real
            # q/G/S matmuls start at 2.4GHz)
            psw = psa.tile([128, 512], f32, tag="psa", name="warm")
            for w in range(12):
                nc.tensor.matmul(psw[:], qbd2[:, 0:128], qbd2[:, 0:512],
                                 start=(w == 0), stop=(w == 11))

            # ---------- q matmuls, both batches packed (N=128) ----------
            # out chunk jc rows = (h,d) of heads (2jc, 2jc+1); cols b*64+q.
            # Evictions build the block-diagonal qbd2: per pair slot of 256
            # cols, [b0 blockdiag 128 | b1 blockdiag 128].
            for jg in range(2):
                ps = psa.tile([128, 512], f32, tag="psa")
                for sub in range(4):
                    jc = jg * 4 + sub
                    for ck in range(CK):
                        nc.tensor.matmul(
                            ps[:, sub * 128:(sub + 1) * 128],
                            wt[:, ck * 1024 + jc * 128: ck * 1024 + (jc + 1) * 128],
                            xqt2[:, ck * 128:(ck + 1) * 128],
                            start=(ck == 0), stop=(ck == CK - 1))
                for sub in range(4):
                    jc = jg * 4 + sub
                    for b in range(BL):
                        base = jc * 256 + b * 128
                        src0 = ps[0:64, sub * 128 + b * 64: sub * 128 + b * 64 + 64]
                        src1 = ps[64:128, sub * 128 + b * 64: sub * 128 + b * 64 + 64]
                        eng = nc.vector if (sub + b) % 2 == 0 else nc.scalar
                        if eng is nc.vector:
                            nc.vector.tensor_copy(qbd2[0:64, base:base + 64], src0)
                            nc.vector.tensor_copy(qbd2[64:128, base + 64:base + 128], src1)
                        else:
                            nc.scalar.activation(qbd2[0:64, base:base + 64], src0, CPY)
                            nc.scalar.activation(qbd2[64:128, base + 64:base + 128], src1, CPY)

            # ---------- G matmuls, both batches packed (N=256) ----------
            gts = []
            for b in range(BL):
                gts.append(gpool.tile([128, CK * 1024], bf16, tag="gt", bufs=2,
                                      name=f"gt{b}"))
            for ck in range(CK):
                for pg in range(4):
                    ps = psa.tile([128, 512], f32, tag="psa")
                    for s2 in range(2):
                        pair = pg * 2 + s2
                        nc.tensor.matmul(
                            ps[:, s2 * 256:(s2 + 1) * 256],
                            wt2[:, ck * 1024 + pair * 128: ck * 1024 + (pair + 1) * 128],
                            qbd2[:, pair * 256:(pair + 1) * 256],
                            start=True, stop=True)
                    ps4 = ps[:].rearrange("p (t b n) -> p t b n", t=2, b=2, n=128)
                    for b in range(BL):
                        dst = gts[b][:, ck * 1024 + pg * 256: ck * 1024 + (pg + 1) * 256]
                        dst3 = dst.rearrange("p (t n) -> p t n", t=2)
                        if (pg + b) % 2 == 0:
                            nc.vector.tensor_copy(dst3, ps4[:, :, b, :])
                        else:
                            nc.scalar.activation(dst3, ps4[:, :, b, :], CPY)

            # ---------- t-loop / epilogue emission helpers ----------
            st = [dict(ptc_prev=None, xnt_prev=None, ut=None, un=None)
                  for _ in range(BL)]

            def emit_block(b, blk, interleave_tail=False, pre=None):
                s = st[b]
                gt = gts[b]
                pacc = paccs[b]
                if blk == 1:
                    s["ut"] = upool.tile([128, CK * 1024], f32, tag="ut",
                                         name=f"ut{b}")
                    s["un"] = gpool.tile([128, CK * 1024], bf16, tag="un",
                                         name=f"un{b}")
                if pre is not None:
                    xtt, xnt = pre
                else:
                    xtt = xpool.tile([128, CK * TB], bf16, tag="xt", bufs=2)
                    nc.sync.dma_start(
                        xtt[:].rearrange("p (t n) -> p t n", t=CK),
                        xt[b, :, blk * TB:(blk + 1) * TB].rearrange(
                            "(t p) n -> p t n", p=128))
                    xnt = xpool.tile([128, TPB * 1024], bf16, tag="xn", bufs=3)
                    nc.sync.dma_start(
                        xnt[:].rearrange("p (t c) -> p t c", t=TPB),
                        xn[b, blk * TB:(blk + 1) * TB, :].rearrange(
                            "(t p) c -> p t c", p=128))

                # S^T + exp into p cache; pacc accumulates p on GpSimd
                ptc = ptp.tile([128, TPB * 1024], bf16, tag="ptc")
                for i in range(TPB):
                    for qh in range(2):
                        stp = psa.tile([128, 512], f32, tag="psa")
                        for ck in range(CK):
                            nc.tensor.matmul(
                                stp[:],
                                xtt[:, ck * TB + i * 128: ck * TB + (i + 1) * 128],
                                gt[:, ck * 1024 + qh * 512: ck * 1024 + (qh + 1) * 512],
                                start=(ck == 0), stop=(ck == CK - 1))
                        pslice = ptc[:, i * 1024 + qh * 512: i * 1024 + (qh + 1) * 512]
                        nc.scalar.activation(pslice, stp[:], EXP, scale=SCALE)
                        pa = pacc[:, qh * 512:(qh + 1) * 512]
                        nc.gpsimd.tensor_add(pa, pslice, pa)

                # u^T accumulation over block pairs, N=512
                if blk % 2 == 0:
                    s["ptc_prev"], s["xnt_prev"] = ptc, xnt
                    return
                ut, un = s["ut"], s["un"]
                last = blk == NBLK - 1
                tail = last and interleave_tail
                for rnd, (qh, cq) in enumerate(
                        [(q, c) for q in range(2) for c in range(2)]):
                    ups = [psu.tile([128, 512], f32, tag="ups",
                                    name=f"ups{b}_{blk}_{qh}_{cq}_{j}")
                           for j in range(4)]
                    # in the very last round, finish each k4's accumulation
                    # before starting the next so its eviction (and the o
                    # matmuls contracting it) unblock ~5us earlier
                    k4_serial = tail and rnd == 3
                    if k4_serial:
                        order = [(k4, half, i) for k4 in range(4)
                                 for half in range(2) for i in range(TPB)]
                    else:
                        order = [(k4, half, i) for half in range(2)
                                 for i in range(TPB) for k4 in range(4)]
                    for k4, half, i in order:
                        pp, xx = ((s["ptc_prev"], s["xnt_prev"]),
                                  (ptc, xnt))[half]
                        ck = cq * 4 + k4
                        nc.tensor.matmul(
                            ups[k4][:],
                            xx[:, i * 1024 + ck * 128: i * 1024 + (ck + 1) * 128],
                            pp[:, i * 1024 + qh * 512: i * 1024 + (qh + 1) * 512],
                            start=(half == 0 and i == 0),
                            stop=(half == 1 and i == TPB - 1))
                    if k4_serial:
                        for k4 in range(4):
                            ck = cq * 4 + k4
                            nc.vector.tensor_add(
                                un[:, ck * 1024 + qh * 512: ck * 1024 + (qh + 1) * 512],
                                ups[k4][:],
                                ut[:, ck * 1024 + qh * 512: ck * 1024 + (qh + 1) * 512])
                        continue
                    for k4 in range(4):
                        ck = cq * 4 + k4
                        dst = ut[:, ck * 1024 + qh * 512: ck * 1024 + (qh + 1) * 512]
                        if blk == 1:
                            nc.vector.tensor_copy(dst, ups[k4][:])
                        elif last:
                            nc.vector.tensor_add(
                                un[:, ck * 1024 + qh * 512: ck * 1024 + (qh + 1) * 512],
                                ups[k4][:], dst)
                        else:
                            nc.vector.tensor_add(dst, ups[k4][:], dst)
                    if tail and rnd == 0:
                        emit_den(b)      # pacc long complete; runs on ACT/DVE
                    if tail and rnd == 2:
                        # o first half for heads 0-7: (qh0,cq0) columns of un
                        # are evicted by now — overlap with the last u round
                        emit_o(b, range(4), 0, 4, start=True, stop=False)
                if tail:
                    emit_o(b, range(4), 4, 8, start=False, stop=True)
                    emit_o(b, range(4, 8), 0, 8, start=True, stop=True)

            def emit_den(b):
                pacc = paccs[b]
                # den replicated on all 128 partitions via all-ones lhsT
                # (bf16 rounding of per-partition partials: ~0.02% on den);
                # fast-approx reciprocal.
                paccb = small.tile([128, HQ], bf16, tag="paccb",
                                   name=f"paccb{b}")
                rdf = small.tile([128, HQ], f32, tag="rdf", name=f"rdf{b}")
                for qh in range(2):
                    nc.scalar.activation(paccb[:, qh * 512:(qh + 1) * 512],
                                         pacc[:, qh * 512:(qh + 1) * 512], CPY)
                    dnp = psu.tile([128, 512], f32, tag="ups",
                                   name=f"dnp{b}_{qh}")
                    nc.tensor.matmul(dnp[:], ones128[:],
                                     paccb[:, qh * 512:(qh + 1) * 512],
                                     start=True, stop=True)
                    nc.vector.reciprocal_approx_fast(
                        rdf[:, qh * 512:(qh + 1) * 512], dnp[:])
                # per-head-pair reciprocal layout for the oT scale:
                # rdo[p, jc*64+qq] = 1/d[(2jc + p//64)*64 + qq]
                # (two strided copies on the scalar engine)
                rdo = small.tile([128, 8 * QL], f32, tag="rdo", name=f"rdo{b}")
                for half in range(2):
                    src = rdf[half * 64:(half + 1) * 64, :].rearrange(
                        "p (j t q) -> p j t q", j=8, t=2)[:, :, half, :]
                    dst = rdo[half * 64:(half + 1) * 64, :].rearrange(
                        "p (j q) -> p j q", j=8)
                    nc.scalar.activation(dst, src, CPY)
                st[b]["rdo"] = rdo
                st[b]["oT"] = small.tile([128, 8 * QL], bf16, tag="oT",
                                         name=f"oT{b}")

            def emit_o(b, jcs, ck_lo, ck_hi, start, stop):
                un = st[b]["un"]
                rdo, oT = st[b]["rdo"], st[b]["oT"]
                ops = st[b].setdefault("ops", {})
                for jc in jcs:  # head pair (2jc, 2jc+1)
                    if start:
                        ops[jc] = psa.tile([128, 512], f32, tag="psa",
                                           name=f"o{b}_{jc}")
                    ps = ops[jc]
                    for sub in range(2):
                        h = jc * 2 + sub
                        ucol = (h // 8) * 512 + (h % 8) * 64
                        for ck in range(ck_lo, ck_hi):
                            nc.tensor.matmul(
                                ps[sub * 64:(sub + 1) * 64, 0:QL],
                                wt3[:, ck * 1024 + h * 64: ck * 1024 + (h + 1) * 64],
                                un[:, ck * 1024 + ucol: ck * 1024 + ucol + 64],
                                start=(start and ck == ck_lo),
                                stop=(stop and ck == ck_hi - 1),
                                tile_position=(0, sub * 64))
                    if stop:
                        nc.vector.tensor_mul(oT[:, jc * QL:(jc + 1) * QL],
                                             ps[:, 0:QL],
                                             rdo[:, jc * QL:(jc + 1) * QL])

            def emit_tail(b):
                oT = st[b]["oT"]
                ys = small.tile([128, C], f32, tag="ys", name=f"ys{b}")
                for half in range(2):
                    ps = psa.tile([128, 512], f32, tag="psa")
                    for jc in range(8):
                        nc.tensor.matmul(
                            ps[0:QL, :],
                            oT[:, jc * QL:(jc + 1) * QL],
                            wt4[:, jc * 1024 + half * 512: jc * 1024 + (half + 1) * 512],
                            start=(jc == 0), stop=(jc == 7))
                    nc.vector.tensor_add(
                        ys[0:QL, half * 512:(half + 1) * 512], ps[0:QL, :],
                        bpf[0:QL, half * 512:(half + 1) * 512])
                nc.sync.dma_start(y[b, :, :], ys[0:QL, :])

            def emit_epilogue(b):
                emit_den(b)
                emit_o(b, range(8), 0, 8, start=True, stop=True)
                emit_tail(b)

            # ---------- main emission: interleave batch boundary ----------
            emit_block(0, 0, pre=(xtt0, xnt0))
            emit_block(0, 1)
            emit_block(0, 2)
            # epilogue weights stream during the t-loop (reuse wpool slots);
            # deferred so they don't steal HBM bandwidth from the startup path
            wt3 = wpool.tile([128, 8 * 1024], bf16, tag="w", name="wt_v")
            nc.sync.dma_start(
                wt3[:].rearrange("p (t c) -> p t c", t=CK),
                wv[:, :].rearrange("(t p) c -> p t c", p=128))
            wt4 = wpool.tile([128, 8 * 1024], bf16, tag="w", name="wt_p")
            nc.sync.dma_start(
                wt4[:].rearrange("p (t c) -> p t c", t=CK),
                wp[:, :].rearrange("(t p) c -> p t c", p=128))
            for blk in range(3, NBLK):
                emit_block(0, blk)
            emit_block(1, 0)
            emit_epilogue(0)      # hidden under b1 block 0/1 matmuls
            for blk in range(1, NBLK - 1):
                emit_block(1, blk)
            # last block: den chain + o matmuls interleaved into the u rounds
            emit_block(1, NBLK - 1, interleave_tail=True)
            emit_tail(1)

    nc.compile()
    return nc


def get_nc():
    if "nc" not in _CACHE:
        _CACHE["nc"] = _build()
    return _CACHE["nc"]


def make_in_maps(x, Wq, Wk, Wv, Wproj, bproj):
    import ml_dtypes
    bf = ml_dtypes.bfloat16
    x = np.ascontiguousarray(x, dtype=np.float32)
    xt32 = np.ascontiguousarray(x.transpose(0, 2, 1))
    xtb = xt32.astype(bf)
    xnb = x.astype(bf)
    wqb = np.ascontiguousarray(np.asarray(Wq, dtype=np.float32).T).astype(bf)
    # wk2[p, ck*1024 + pair*128 + m] = Wk[pair*128 + p, ck*128 + m]
    wkb = np.ascontiguousarray(
        np.asarray(Wk, dtype=np.float32).reshape(8, 128, 8, 128)
        .transpose(1, 2, 0, 3).reshape(128, 8 * 1024)).astype(bf)
    wvb = np.ascontiguousarray(np.asarray(Wv, dtype=np.float32).T).astype(bf)
    wpb = np.ascontiguousarray(np.asarray(Wproj, dtype=np.float32).T).astype(bf)
    bpf = np.ascontiguousarray(
        np.asarray(bproj, dtype=np.float32).reshape(1, C)).astype(bf)
    in_maps = []
    for core in range(8):
        s = slice(core * BL, (core + 1) * BL)
        # xq2[c, b*64+q] = x^T[b, c, q] for the core's two batches
        xq2 = np.concatenate([xt32[core * BL + b, :, 0:QL] for b in range(BL)],
                             axis=1).astype(bf)
        in_maps.append({
            "xn": np.ascontiguousarray(xnb[s]),
            "xt": np.ascontiguousarray(xtb[s]),
            "xq2": np.ascontiguousarray(xq2),
            "wq": wqb, "wk2": wkb, "wv": wvb, "wp": wpb, "bp": bpf,
        })
    return in_maps


def kernel(x, Wq, Wk, Wv, Wproj, bproj):
    from concourse import bass_utils
    nc = get_nc()
    in_maps = make_in_maps(x, Wq, Wk, Wv, Wproj, bproj)
    res = bass_utils.run_bass_kernel_spmd(nc, in_maps, core_ids=list(range(8)))
    out = np.concatenate([res.results[i]["y"] for i in range(8)], axis=0)
    return out.astype(np.float32)


# revision 31
# speedup vs baseline: 1.0013x; 1.0013x over previous
"""Trainium2 Bass kernel for nn_Attention (B=16,N=4096,C=1024,H=16,HD=64,Q=64).

Data-parallel over B across 8 NeuronCores (2 batches/core). Per batch the
attention is reassociated so no k/v tensors are materialized and no on-chip
transposes are needed:

  q^T = Wq @ x_q^T                      [(h,d)=1024, 64]
  G_h^T = Wk_h^T @ q_h                  G^T: [c=1024, (h,q)=1024]
  S^T   = x @ G^T   (per t-tile)        [t, (h,q)]   (contract c)
  p^T   = exp(S^T / 8)                  (softmax w/o max-sub: scores ~ +-5)
  u^T   = x^T(nat) @ p^T  (accum t)     [c, (h,q)]   (contract t)
  den   = ones @ pacc     (pacc: GpSimd p-sum over t)
  o_h^T = (Wv_h^T)^T @ u_h^T, scaled by 1/den at PSUM eviction
  y     = o^T.T @ Wproj^T + b           [64, 1024]   (contract (h,d))

Optimizations over the first working version:
  - startup: consolidated/prioritized DMAs (order xq2 -> wq -> x-block0 ->
    wk -> xnt0; wv/wp deferred past block 2) with per-ck granularity so the
    q/G matmuls pipeline with the weight stream; wk pre-tiled on host so its
    DMA runs full-bandwidth 2KB lines.
  - q matmuls packed across both batches (N=128), G matmuls packed (N=256),
    PSUM evictions split across DVE and ACT.
  - batch 1's block 0 is emitted before batch 0's epilogue so the PE never
    drains at the batch boundary (keeps HAM at K=8/8).
  - softmax denominators: bf16 pacc copy -> all-ones matmul replicates den
    across partitions (no GpSimd broadcast), reciprocal_approx_fast, rdo via
    two strided ACT copies; dn PSUM tiles live in the u pool so o matmuls
    keep all 4 psa banks.
  - last batch's tail: den chain + o matmuls interleaved into the final u
    rounds (o first-half runs while the last u round computes), so only the
    last 4 PSUM evictions + o second-half + proj remain after the last u
    matmul.
"""
import os
import numpy as np

B, N, C = 16, 4096, 1024
H, HD, QL = 16, 64, 64
BL = B // 8           # batches per core
CK = C // 128         # 8 c-tiles
TB = 512              # tokens per t-block
NBLK = N // TB        # 8 blocks
TPB = TB // 128       # 4 t-tiles per block
HQ = H * QL           # 1024
SCALE = HD ** -0.5

_CACHE = {}


def _build():
    import concourse.bass as bass
    import concourse.tile as tile
    from concourse import bacc, mybir

    f32 = mybir.dt.float32
    bf16 = mybir.dt.bfloat16
    EXP = mybir.ActivationFunctionType.Exp
    CPY = mybir.ActivationFunctionType.Copy

    nc = bacc.Bacc("TRN2", target_bir_lowering=False, debug=False, num_devices=8)
    xn = nc.dram_tensor("xn", [BL, N, C], bf16, kind="ExternalInput").ap()
    xt = nc.dram_tensor("xt", [BL, C, N], bf16, kind="ExternalInput").ap()
    wq = nc.dram_tensor("wq", [C, C], bf16, kind="ExternalInput").ap()   # Wq^T
    # Wk pre-tiled on host: wk2[p, ck*1024+pair*128+m] = Wk[pair*128+p, ck*128+m]
    wk2 = nc.dram_tensor("wk2", [128, CK * 1024], bf16, kind="ExternalInput").ap()
    wv = nc.dram_tensor("wv", [C, C], bf16, kind="ExternalInput").ap()   # Wv^T
    wp = nc.dram_tensor("wp", [C, C], bf16, kind="ExternalInput").ap()   # Wproj^T
    bp = nc.dram_tensor("bp", [1, C], bf16, kind="ExternalInput").ap()
    # xq2[c, b*64+q] = x[b, q, c] for this core's two batches
    xq2 = nc.dram_tensor("xq2", [C, 2 * QL], bf16, kind="ExternalInput").ap()
    y = nc.dram_tensor("y", [BL, QL, C], f32, kind="ExternalOutput").ap()

    with tile.TileContext(nc) as tc:
        with (
            tc.tile_pool(name="wpool", bufs=2) as wpool,
            tc.tile_pool(name="xpool", bufs=2) as xpool,
            tc.tile_pool(name="gpool", bufs=1) as gpool,
            tc.tile_pool(name="upool", bufs=1) as upool,
            tc.tile_pool(name="small", bufs=1) as small,
            tc.tile_pool(name="ptp", bufs=3) as ptp,
            tc.tile_pool(name="psa", bufs=4, space="PSUM") as psa,
            tc.tile_pool(name="psu", bufs=4, space="PSUM") as psu,
        ):
            # ---------- memsets first: no DMA deps, run under the launch ----
            ones128 = small.tile([128, 128], bf16, tag="ones128")
            nc.gpsimd.memset(ones128[:], 1.0)
            qbd2 = small.tile([128, 8 * 256], bf16, tag="qbd2")
            nc.gpsimd.memset(qbd2[:], 0.0)
            paccs = []
            for b in range(BL):
                pacc = small.tile([128, HQ], f32, tag="pacc", bufs=2,
                                  name=f"pacc{b}")
                nc.gpsimd.memset(pacc[:], 0.0)
                paccs.append(pacc)

            # ---------- startup DMAs in dependency-priority order ----------
            xqt2 = small.tile([128, CK * 128], bf16, tag="xqt")
            nc.sync.dma_start(
                xqt2[:].rearrange("p (t q) -> p t q", t=CK),
                xq2[:, :].rearrange("(t p) q -> p t q", p=128))
            # Wq^T per c-tile so q matmuls pipeline with the stream
            wt = wpool.tile([128, 8 * 1024], bf16, tag="w", name="wt_q")
            for ck in range(CK):
                nc.sync.dma_start(wt[:, ck * 1024:(ck + 1) * 1024],
                                  wq[ck * 128:(ck + 1) * 128, :])
            # Wk host-pre-tiled column-tile-major: wt2[p, ck*1024+pair*128+m]
            #   = wk[pair*128 + p, ck*128 + m]; G's ck-tiles unblock in order.
            # (PE runs in program order, so ALL G matmuls precede the first S
            # matmul — wk must land before block-0 x^T, not after)
            wt2 = wpool.tile([128, 8 * 1024], bf16, tag="w", name="wt_k")
            for ck in range(CK):
                nc.sync.dma_start(wt2[:, ck * 1024:(ck + 1) * 1024],
                                    wk2[:, ck * 1024:(ck + 1) * 1024])
            xtt0 = xpool.tile([128, CK * TB], bf16, tag="xt", bufs=2)
            nc.sync.dma_start(
                xtt0[:].rearrange("p (t n) -> p t n", t=CK),
                xt[0, :, 0:TB].rearrange("(t p) n -> p t n", p=128))
            xnt0 = xpool.tile([128, TPB * 1024], bf16, tag="xn", bufs=3)
            nc.sync.dma_start(
                xnt0[:].rearrange("p (t c) -> p t c", t=TPB),
                xn[0, 0:TB, :].rearrange("(t p) c -> p t c", p=128))
            bps = small.tile([128, C], bf16, tag="bps")
            nc.sync.dma_start(bps[0:1, :], bp[:, :])
            bpf = small.tile([128, C], bf16, tag="bpf")
            nc.gpsimd.partition_broadcast(bpf[:], bps[0:1, :])

            # ---------- PE warm-up: ~12 junk matmuls under the DMA wait ----
            # (HAM releases the 4/8 clock throttle after ~3.4us of sustained
            # PE activity; this burst runs while wq streams so the real
            # q/G/S matmuls start at 2.4GHz)
            psw = psa.tile([128, 512], f32, tag="psa", name="warm")
            for w in range(12):
                nc.tensor.matmul(psw[:], qbd2[:, 0:128], qbd2[:, 0:512],
                                 start=(w == 0), stop=(w == 11))

            # ---------- q matmuls, both batches packed (N=128) ----------
            # out chunk jc rows = (h,d) of heads (2jc, 2jc+1); cols b*64+q.
            # Evictions build the block-diagonal qbd2: per pair slot of 256
            # cols, [b0 blockdiag 128 | b1 blockdiag 128].
            for jg in range(2):
                ps = psa.tile([128, 512], f32, tag="psa")
                for sub in range(4):
                    jc = jg * 4 + sub
                    for ck in range(CK):
                        nc.tensor.matmul(
                            ps[:, sub * 128:(sub + 1) * 128],
                            wt[:, ck * 1024 + jc * 128: ck * 1024 + (jc + 1) * 128],
                            xqt2[:, ck * 128:(ck + 1) * 128],
                            start=(ck == 0), stop=(ck == CK - 1))
                for sub in range(4):
                    jc = jg * 4 + sub
                    for b in range(BL):
                        base = jc * 256 + b * 128
                        src0 = ps[0:64, sub * 128 + b * 64: sub * 128 + b * 64 + 64]
                        src1 = ps[64:128, sub * 128 + b * 64: sub * 128 + b * 64 + 64]
                        eng = nc.vector if (sub + b) % 2 == 0 else nc.scalar
                        if eng is nc.vector:
                            nc.vector.tensor_copy(qbd2[0:64, base:base + 64], src0)
                            nc.vector.tensor_copy(qbd2[64:128, base + 64:base + 128], src1)
                        else:
                            nc.scalar.activation(qbd2[0:64, base:base + 64], src0, CPY)
                            nc.scalar.activation(qbd2[64:128, base + 64:base + 128], src1, CPY)

            # ---------- G matmuls, both batches packed (N=256) ----------
            gts = []
            for b in range(BL):
                gts.append(gpool.tile([128, CK * 1024], bf16, tag="gt", bufs=2,
                                      name=f"gt{b}"))
            for ck in range(CK):
                for pg in range(4):
                    ps = psa.tile([128, 512], f32, tag="psa")
                    for s2 in range(2):
                        pair = pg * 2 + s2
                        nc.tensor.matmul(
                            ps[:, s2 * 256:(s2 + 1) * 256],
                            wt2[:, ck * 1024 + pair * 128: ck * 1024 + (pair + 1) * 128],
                            qbd2[:, pair * 256:(pair + 1) * 256],
                            start=True, stop=True)
                    ps4 = ps[:].rearrange("p (t b n) -> p t b n", t=2, b=2, n=128)
                    for b in range(BL):
                        dst = gts[b][:, ck * 1024 + pg * 256: ck * 1024 + (pg + 1) * 256]
                        dst3 = dst.rearrange("p (t n) -> p t n", t=2)
                        if (pg + b) % 2 == 0:
                            nc.vector.tensor_copy(dst3, ps4[:, :, b, :])
                        else:
                            nc.scalar.activation(dst3, ps4[:, :, b, :], CPY)

            # ---------- t-loop / epilogue emission helpers ----------
            st = [dict(ptc_prev=None, xnt_prev=None, ut=None, un=None)
                  for _ in range(BL)]

            def emit_block(b, blk, interleave_tail=False, pre=None):
                s = st[b]
                gt = gts[b]
                pacc = paccs[b]
                if blk == 1:
                    s["ut"] = upool.tile([128, CK * 1024], f32, tag="ut",
                                         name=f"ut{b}")
                    s["un"] = gpool.tile([128, CK * 1024], bf16, tag="un",
                                         name=f"un{b}")
                if pre is not None:
                    xtt, xnt = pre
                else:
                    xtt = xpool.tile([128, CK * TB], bf16, tag="xt", bufs=2)
                    nc.sync.dma_start(
                        xtt[:].rearrange("p (t n) -> p t n", t=CK),
                        xt[b, :, blk * TB:(blk + 1) * TB].rearrange(
                            "(t p) n -> p t n", p=128))
                    xnt = xpool.tile([128, TPB * 1024], bf16, tag="xn", bufs=3)
                    nc.sync.dma_start(
                        xnt[:].rearrange("p (t c) -> p t c", t=TPB),
                        xn[b, blk * TB:(blk + 1) * TB, :].rearrange(
                            "(t p) c -> p t c", p=128))

                # S^T + exp into p cache; pacc accumulates p on GpSimd
                ptc = ptp.tile([128, TPB * 1024], bf16, tag="ptc")
                for i in range(TPB):
                    for qh in range(2):
                        stp = psa.tile([128, 512], f32, tag="psa")
                        for ck in range(CK):
                            nc.tensor.matmul(
                                stp[:],
                                xtt[:, ck * TB + i * 128: ck * TB + (i + 1) * 128],
                                gt[:, ck * 1024 + qh * 512: ck * 1024 + (qh + 1) * 512],
                                start=(ck == 0), stop=(ck == CK - 1))
                        pslice = ptc[:, i * 1024 + qh * 512: i * 1024 + (qh + 1) * 512]
                        nc.scalar.activation(pslice, stp[:], EXP, scale=SCALE)
                        pa = pacc[:, qh * 512:(qh + 1) * 512]
                        nc.gpsimd.tensor_add(pa, pslice, pa)

                # u^T accumulation over block pairs, N=512
                if blk % 2 == 0:
                    s["ptc_prev"], s["xnt_prev"] = ptc, xnt
                    return
                ut, un = s["ut"], s["un"]
                last = blk == NBLK - 1
                tail = last and interleave_tail
                for rnd, (qh, cq) in enumerate(
                        [(q, c) for q in range(2) for c in range(2)]):
                    ups = [psu.tile([128, 512], f32, tag="ups",
                                    name=f"ups{b}_{blk}_{qh}_{cq}_{j}")
                           for j in range(4)]
                    # in the very last round, finish each k4's accumulation
                    # before starting the next so its eviction (and the o
                    # matmuls contracting it) unblock ~5us earlier
                    k4_serial = tail and rnd == 3
                    if k4_serial:
                        order = [(k4, half, i) for k4 in range(4)
                                 for half in range(2) for i in range(TPB)]
                    else:
                        order = [(k4, half, i) for half in range(2)
                                 for i in range(TPB) for k4 in range(4)]
                    for k4, half, i in order:
                        pp, xx = ((s["ptc_prev"], s["xnt_prev"]),
                                  (ptc, xnt))[half]
                        ck = cq * 4 + k4
                        nc.tensor.matmul(
                            ups[k4][:],
                            xx[:, i * 1024 + ck * 128: i * 1024 + (ck + 1) * 128],
                            pp[:, i * 1024 + qh * 512: i * 1024 + (qh + 1) * 512],
                            start=(half == 0 and i == 0),
                            stop=(half == 1 and i == TPB - 1))
                    if k4_serial:
                        for k4 in range(4):
                            ck = cq * 4 + k4
                            nc.vector.tensor_add(
                                un[:, ck * 1024 + qh * 512: ck * 1024 + (qh + 1) * 512],
                                ups[k4][:],
                                ut[:, ck * 1024 + qh * 512: ck * 1024 + (qh + 1) * 512])
                        continue
                    for k4 in range(4):
                        ck = cq * 4 + k4
                        dst = ut[:, ck * 1024 + qh * 512: ck * 1024 + (qh + 1) * 512]
                        if blk == 1:
                            nc.vector.tensor_copy(dst, ups[k4][:])
                        elif last:
                            nc.vector.tensor_add(
                                un[:, ck * 1024 + qh * 512: ck * 1024 + (qh + 1) * 512],
                                ups[k4][:], dst)
                        else:
                            nc.vector.tensor_add(dst, ups[k4][:], dst)
                    if tail and rnd == 0:
                        emit_den(b)      # pacc long complete; runs on ACT/DVE
                    if tail and rnd == 2:
                        # o first half for heads 0-7: (qh0,cq0) columns of un
                        # are evicted by now — overlap with the last u round
                        emit_o(b, range(4), 0, 4, start=True, stop=False)
                if tail:
                    emit_o(b, range(4), 4, 8, start=False, stop=True)
                    emit_o(b, range(4, 8), 0, 8, start=True, stop=True)

            def emit_den(b):
                pacc = paccs[b]
                # den replicated on all 128 partitions via all-ones lhsT
                # (bf16 rounding of per-partition partials: ~0.02% on den);
                # fast-approx reciprocal.
                paccb = small.tile([128, HQ], bf16, tag="paccb",
                                   name=f"paccb{b}")
                rdf = small.tile([128, HQ], f32, tag="rdf", name=f"rdf{b}")
                for qh in range(2):
                    nc.scalar.activation(paccb[:, qh * 512:(qh + 1) * 512],
                                         pacc[:, qh * 512:(qh + 1) * 512], CPY)
                    dnp = psu.tile([128, 512], f32, tag="ups",
                                   name=f"dnp{b}_{qh}")
                    nc.tensor.matmul(dnp[:], ones128[:],
                                     paccb[:, qh * 512:(qh + 1) * 512],
                                     start=True, stop=True)
                    nc.vector.reciprocal_approx_fast(
                        rdf[:, qh * 512:(qh + 1) * 512], dnp[:])
                # per-head-pair reciprocal layout for the oT scale:
                # rdo[p, jc*64+qq] = 1/d[(2jc + p//64)*64 + qq]
                # (two strided copies on the scalar engine)
                rdo = small.tile([128, 8 * QL], f32, tag="rdo", name=f"rdo{b}")
                for half in range(2):
                    src = rdf[half * 64:(half + 1) * 64, :].rearrange(
                        "p (j t q) -> p j t q", j=8, t=2)[:, :, half, :]
                    dst = rdo[half * 64:(half + 1) * 64, :].rearrange(
                        "p (j q) -> p j q", j=8)
                    nc.scalar.activation(dst, src, CPY)
                st[b]["rdo"] = rdo
                st[b]["oT"] = small.tile([128, 8 * QL], bf16, tag="oT",
                                         name=f"oT{b}")

            def emit_o(b, jcs, ck_lo, ck_hi, start, stop):
                un = st[b]["un"]
                rdo, oT = st[b]["rdo"], st[b]["oT"]
                ops = st[b].setdefault("ops", {})
                for jc in jcs:  # head pair (2jc, 2jc+1)
                    if start:
                        ops[jc] = psa.tile([128, 512], f32, tag="psa",
                                           name=f"o{b}_{jc}")
                    ps = ops[jc]
                    for sub in range(2):
                        h = jc * 2 + sub
                        ucol = (h // 8) * 512 + (h % 8) * 64
                        for ck in range(ck_lo, ck_hi):
                            nc.tensor.matmul(
                                ps[sub * 64:(sub + 1) * 64, 0:QL],
                                wt3[:, ck * 1024 + h * 64: ck * 1024 + (h + 1) * 64],
                                un[:, ck * 1024 + ucol: ck * 1024 + ucol + 64],
                                start=(start and ck == ck_lo),
                                stop=(stop and ck == ck_hi - 1),
                                tile_position=(0, sub * 64))
                    if stop:
                        nc.vector.tensor_mul(oT[:, jc * QL:(jc + 1) * QL],
                                             ps[:, 0:QL],
                                             rdo[:, jc * QL:(jc + 1) * QL])

            def emit_tail(b):
                oT = st[b]["oT"]
                ys = small.tile([128, C], f32, tag="ys", name=f"ys{b}")
                for half in range(2):
                    ps = psa.tile([128, 512], f32, tag="psa")
                    for jc in range(8):
                        nc.tensor.matmul(
                            ps[0:QL, :],
                            oT[:, jc * QL:(jc + 1) * QL],
                            wt4[:, jc * 1024 + half * 512: jc * 1024 + (half + 1) * 512],
                            start=(jc == 0), stop=(jc == 7))
                    nc.vector.tensor_add(
                        ys[0:QL, half * 512:(half + 1) * 512], ps[0:QL, :],
                        bpf[0:QL, half * 512:(half + 1) * 512])
                nc.sync.dma_start(y[b, :, :], ys[0:QL, :])

            def emit_epilogue(b):
                emit_den(b)
                emit_o(b, range(8), 0, 8, start=True, stop=True)
                emit_tail(b)

            # ---------- main emission: interleave batch boundary ----------
            emit_block(0, 0, pre=(xtt0, xnt0))
            emit_block(0, 1)
            emit_block(0, 2)
            # epilogue weights stream during the t-loop (reuse wpool slots);
            # deferred so they don't steal HBM bandwidth from the startup path
            wt3 = wpool.tile([128, 8 * 1024], bf16, tag="w", name="wt_v")
            nc.sync.dma_start(
                wt3[:].rearrange("p (t c) -> p t c", t=CK),
                wv[:, :].rearrange("(t p) c -> p t c", p=128))
            wt4 = wpool.tile([128, 8 * 1024], bf16, tag="w", name="wt_p")
            nc.sync.dma_start(
                wt4[:].rearrange("p (t c) -> p t c", t=CK),
                wp[:, :].rearrange("(t p) c -> p t c", p=128))
            for blk in range(3, NBLK):
                emit_block(0, blk)
            emit_block(1, 0)
            emit_epilogue(0)      # hidden under b1 block 0/1 matmuls
            for blk in range(1, NBLK - 1):
                emit_block(1, blk)
            # last block: den chain + o matmuls interleaved into the u rounds
            emit_block(1, NBLK - 1, interleave_tail=True)
            emit_tail(1)

    nc.compile()
    return nc


def get_nc():
    if "nc" not in _CACHE:
        _CACHE["nc"] = _build()
    return _CACHE["nc"]


def make_in_maps(x, Wq, Wk, Wv, Wproj, bproj):
    import ml_dtypes
    bf = ml_dtypes.bfloat16
    x = np.ascontiguousarray(x, dtype=np.float32)
    xt32 = np.ascontiguousarray(x.transpose(0, 2, 1))
    xtb = xt32.astype(bf)
    xnb = x.astype(bf)
    wqb = np.ascontiguousarray(np.asarray(Wq, dtype=np.float32).T).astype(bf)
    # wk2[p, ck*1024 + pair*128 + m] = Wk[pair*128 + p, ck*128 + m]
    wkb = np.ascontiguousarray(
        np.asarray(Wk, dtype=np.float32).reshape(8, 128, 8, 128)
        .transpose(1, 2, 0, 3).reshape(128, 8 * 1024)).astype(bf)
    wvb = np.ascontiguousarray(np.asarray(Wv, dtype=np.float32).T).astype(bf)
    wpb = np.ascontiguousarray(np.asarray(Wproj, dtype=np.float32).T).astype(bf)
    bpf = np.ascontiguousarray(
        np.asarray(bproj, dtype=np.float32).reshape(1, C)).astype(bf)
    in_maps = []
    for core in range(8):
        s = slice(core * BL, (core + 1) * BL)
        # xq2[c, b*64+q] = x^T[b, c, q] for the core's two batches
        xq2 = np.concatenate([xt32[core * BL + b, :, 0:QL] for b in range(BL)],
                             axis=1).astype(bf)
        in_maps.append({
            "xn": np.ascontiguousarray(xnb[s]),
            "xt": np.ascontiguousarray(xtb[s]),
            "xq2": np.ascontiguousarray(xq2),
            "wq": wqb, "wk2": wkb, "wv": wvb, "wp": wpb, "bp": bpf,
        })
    return in_maps


def kernel(x, Wq, Wk, Wv, Wproj, bproj):
    from concourse import bass_utils
    nc = get_nc()
    in_maps = make_in_maps(x, Wq, Wk, Wv, Wproj, bproj)
    res = bass_utils.run_bass_kernel_spmd(nc, in_maps, core_ids=list(range(8)))
    out = np.concatenate([res.results[i]["y"] for i in range(8)], axis=0)
    return out.astype(np.float32)


# revision 32
# speedup vs baseline: 1.0027x; 1.0014x over previous
"""Trainium2 Bass kernel for nn_Attention (B=16,N=4096,C=1024,H=16,HD=64,Q=64).

Data-parallel over B across 8 NeuronCores (2 batches/core). Per batch the
attention is reassociated so no k/v tensors are materialized and no on-chip
transposes are needed:

  q^T = Wq @ x_q^T                      [(h,d)=1024, 64]
  G_h^T = Wk_h^T @ q_h                  G^T: [c=1024, (h,q)=1024]
  S^T   = x @ G^T   (per t-tile)        [t, (h,q)]   (contract c)
  p^T   = exp(S^T / 8)                  (softmax w/o max-sub: scores ~ +-5)
  u^T   = x^T(nat) @ p^T  (accum t)     [c, (h,q)]   (contract t)
  den   = ones @ pacc     (pacc: GpSimd p-sum over t)
  o_h^T = (Wv_h^T)^T @ u_h^T, scaled by 1/den at PSUM eviction
  y     = o^T.T @ Wproj^T + b           [64, 1024]   (contract (h,d))

Optimizations over the first working version:
  - startup: consolidated/prioritized DMAs (order xq2 -> wq -> x-block0 ->
    wk -> xnt0; wv/wp deferred past block 2) with per-ck granularity so the
    q/G matmuls pipeline with the weight stream; wk pre-tiled on host so its
    DMA runs full-bandwidth 2KB lines.
  - q matmuls packed across both batches (N=128), G matmuls packed (N=256),
    PSUM evictions split across DVE and ACT.
  - batch 1's block 0 is emitted before batch 0's epilogue so the PE never
    drains at the batch boundary (keeps HAM at K=8/8).
  - softmax denominators: bf16 pacc copy -> all-ones matmul replicates den
    across partitions (no GpSimd broadcast), reciprocal_approx_fast, rdo via
    two strided ACT copies; dn PSUM tiles live in the u pool so o matmuls
    keep all 4 psa banks.
  - last batch's tail: den chain + o matmuls interleaved into the final u
    rounds (o first-half runs while the last u round computes), so only the
    last 4 PSUM evictions + o second-half + proj remain after the last u
    matmul.
"""
import os
import numpy as np

B, N, C = 16, 4096, 1024
H, HD, QL = 16, 64, 64
BL = B // 8           # batches per core
CK = C // 128         # 8 c-tiles
TB = 512              # tokens per t-block
NBLK = N // TB        # 8 blocks
TPB = TB // 128       # 4 t-tiles per block
HQ = H * QL           # 1024
SCALE = HD ** -0.5

_CACHE = {}


def _build():
    import concourse.bass as bass
    import concourse.tile as tile
    from concourse import bacc, mybir

    f32 = mybir.dt.float32
    bf16 = mybir.dt.bfloat16
    EXP = mybir.ActivationFunctionType.Exp
    CPY = mybir.ActivationFunctionType.Copy

    nc = bacc.Bacc("TRN2", target_bir_lowering=False, debug=False, num_devices=8)
    xn = nc.dram_tensor("xn", [BL, N, C], bf16, kind="ExternalInput").ap()
    xt = nc.dram_tensor("xt", [BL, C, N], bf16, kind="ExternalInput").ap()
    wq = nc.dram_tensor("wq", [C, C], bf16, kind="ExternalInput").ap()   # Wq^T
    # Wk pre-tiled on host: wk2[p, ck*1024+pair*128+m] = Wk[pair*128+p, ck*128+m]
    wk2 = nc.dram_tensor("wk2", [128, CK * 1024], bf16, kind="ExternalInput").ap()
    wv = nc.dram_tensor("wv", [C, C], bf16, kind="ExternalInput").ap()   # Wv^T
    wp = nc.dram_tensor("wp", [C, C], bf16, kind="ExternalInput").ap()   # Wproj^T
    bp = nc.dram_tensor("bp", [1, C], bf16, kind="ExternalInput").ap()
    # xq2[c, b*64+q] = x[b, q, c] for this core's two batches
    xq2 = nc.dram_tensor("xq2", [C, 2 * QL], bf16, kind="ExternalInput").ap()
    y = nc.dram_tensor("y", [BL, QL, C], f32, kind="ExternalOutput").ap()

    with tile.TileContext(nc) as tc:
        with (
            tc.tile_pool(name="wpool", bufs=2) as wpool,
            tc.tile_pool(name="xpool", bufs=2) as xpool,
            tc.tile_pool(name="gpool", bufs=1) as gpool,
            tc.tile_pool(name="upool", bufs=1) as upool,
            tc.tile_pool(name="small", bufs=1) as small,
            tc.tile_pool(name="ptp", bufs=3) as ptp,
            tc.tile_pool(name="psa", bufs=4, space="PSUM") as psa,
            tc.tile_pool(name="psu", bufs=4, space="PSUM") as psu,
        ):
            # ---------- memsets first: no DMA deps, run under the launch ----
            ones128 = small.tile([128, 128], bf16, tag="ones128")
            nc.gpsimd.memset(ones128[:], 1.0)
            qbd2 = small.tile([128, 8 * 256], bf16, tag="qbd2")
            nc.gpsimd.memset(qbd2[:], 0.0)
            paccs = []
            for b in range(BL):
                pacc = small.tile([128, HQ], f32, tag="pacc", bufs=2,
                                  name=f"pacc{b}")
                nc.gpsimd.memset(pacc[:], 0.0)
                paccs.append(pacc)

            # ---------- startup DMAs in dependency-priority order ----------
            xqt2 = small.tile([128, CK * 128], bf16, tag="xqt")
            nc.sync.dma_start(
                xqt2[:].rearrange("p (t q) -> p t q", t=CK),
                xq2[:, :].rearrange("(t p) q -> p t q", p=128))
            # Wq^T per c-tile so q matmuls pipeline with the stream
            wt = wpool.tile([128, 8 * 1024], bf16, tag="w", name="wt_q")
            for ck in range(CK):
                nc.sync.dma_start(wt[:, ck * 1024:(ck + 1) * 1024],
                                  wq[ck * 128:(ck + 1) * 128, :])
            # Wk host-pre-tiled column-tile-major: wt2[p, ck*1024+pair*128+m]
            #   = wk[pair*128 + p, ck*128 + m]; G's ck-tiles unblock in order.
            # (PE runs in program order, so ALL G matmuls precede the first S
            # matmul — wk must land before block-0 x^T, not after)
            wt2 = wpool.tile([128, 8 * 1024], bf16, tag="w", name="wt_k")
            for ck in range(CK):
                nc.sync.dma_start(wt2[:, ck * 1024:(ck + 1) * 1024],
                                    wk2[:, ck * 1024:(ck + 1) * 1024])
            xtt0 = xpool.tile([128, CK * TB], bf16, tag="xt", bufs=2)
            nc.sync.dma_start(
                xtt0[:].rearrange("p (t n) -> p t n", t=CK),
                xt[0, :, 0:TB].rearrange("(t p) n -> p t n", p=128))
            xnt0 = xpool.tile([128, TPB * 1024], bf16, tag="xn", bufs=3)
            nc.sync.dma_start(
                xnt0[:].rearrange("p (t c) -> p t c", t=TPB),
                xn[0, 0:TB, :].rearrange("(t p) c -> p t c", p=128))
            bps = small.tile([128, C], bf16, tag="bps")
            nc.sync.dma_start(bps[0:1, :], bp[:, :])
            bpf = small.tile([128, C], bf16, tag="bpf")
            nc.gpsimd.partition_broadcast(bpf[:], bps[0:1, :])

            # ---------- PE warm-up: ~12 junk matmuls under the DMA wait ----
            # (HAM releases the 4/8 clock throttle after ~3.4us of sustained
            # PE activity; this burst runs while wq streams so the real
            # q/G/S matmuls start at 2.4GHz)
            psw = psa.tile([128, 512], f32, tag="psa", name="warm")
            for w in range(12):
                nc.tensor.matmul(psw[:], qbd2[:, 0:128], qbd2[:, 0:512],
                                 start=(w == 0), stop=(w == 11))

            # ---------- q matmuls, both batches packed (N=128) ----------
            # out chunk jc rows = (h,d) of heads (2jc, 2jc+1); cols b*64+q.
            # Evictions build the block-diagonal qbd2: per pair slot of 256
            # cols, [b0 blockdiag 128 | b1 blockdiag 128].
            for jg in range(2):
                ps = psa.tile([128, 512], f32, tag="psa")
                for sub in range(4):
                    jc = jg * 4 + sub
                    for ck in range(CK):
                        nc.tensor.matmul(
                            ps[:, sub * 128:(sub + 1) * 128],
                            wt[:, ck * 1024 + jc * 128: ck * 1024 + (jc + 1) * 128],
                            xqt2[:, ck * 128:(ck + 1) * 128],
                            start=(ck == 0), stop=(ck == CK - 1))
                for sub in range(4):
                    jc = jg * 4 + sub
                    for b in range(BL):
                        base = jc * 256 + b * 128
                        src0 = ps[0:64, sub * 128 + b * 64: sub * 128 + b * 64 + 64]
                        src1 = ps[64:128, sub * 128 + b * 64: sub * 128 + b * 64 + 64]
                        eng = nc.vector if (sub + b) % 2 == 0 else nc.scalar
                        if eng is nc.vector:
                            nc.vector.tensor_copy(qbd2[0:64, base:base + 64], src0)
                            nc.vector.tensor_copy(qbd2[64:128, base + 64:base + 128], src1)
                        else:
                            nc.scalar.activation(qbd2[0:64, base:base + 64], src0, CPY)
                            nc.scalar.activation(qbd2[64:128, base + 64:base + 128], src1, CPY)

            # ---------- G matmuls, both batches packed (N=256) ----------
            gts = []
            for b in range(BL):
                gts.append(gpool.tile([128, CK * 1024], bf16, tag="gt", bufs=2,
                                      name=f"gt{b}"))
            for ck in range(CK):
                for pg in range(4):
                    ps = psa.tile([128, 512], f32, tag="psa")
                    for s2 in range(2):
                        pair = pg * 2 + s2
                        nc.tensor.matmul(
                            ps[:, s2 * 256:(s2 + 1) * 256],
                            wt2[:, ck * 1024 + pair * 128: ck * 1024 + (pair + 1) * 128],
                            qbd2[:, pair * 256:(pair + 1) * 256],
                            start=True, stop=True)
                    ps4 = ps[:].rearrange("p (t b n) -> p t b n", t=2, b=2, n=128)
                    for b in range(BL):
                        dst = gts[b][:, ck * 1024 + pg * 256: ck * 1024 + (pg + 1) * 256]
                        dst3 = dst.rearrange("p (t n) -> p t n", t=2)
                        if (pg + b) % 2 == 0:
                            nc.vector.tensor_copy(dst3, ps4[:, :, b, :])
                        else:
                            nc.scalar.activation(dst3, ps4[:, :, b, :], CPY)

            # ---------- t-loop / epilogue emission helpers ----------
            st = [dict(ptc_prev=None, xnt_prev=None, ut=None, un=None)
                  for _ in range(BL)]

            def emit_block(b, blk, interleave_tail=False, pre=None):
                s = st[b]
                gt = gts[b]
                pacc = paccs[b]
                if blk == 1:
                    s["ut"] = upool.tile([128, CK * 1024], f32, tag="ut",
                                         name=f"ut{b}")
                    s["un"] = gpool.tile([128, CK * 1024], bf16, tag="un",
                                         name=f"un{b}")
                if pre is not None:
                    xtt, xnt = pre
                else:
                    xtt = xpool.tile([128, CK * TB], bf16, tag="xt", bufs=2)
                    nc.sync.dma_start(
                        xtt[:].rearrange("p (t n) -> p t n", t=CK),
                        xt[b, :, blk * TB:(blk + 1) * TB].rearrange(
                            "(t p) n -> p t n", p=128))
                    xnt = xpool.tile([128, TPB * 1024], bf16, tag="xn", bufs=3)
                    nc.sync.dma_start(
                        xnt[:].rearrange("p (t c) -> p t c", t=TPB),
                        xn[b, blk * TB:(blk + 1) * TB, :].rearrange(
                            "(t p) c -> p t c", p=128))

                # S^T + exp into p cache; pacc accumulates p on GpSimd
                ptc = ptp.tile([128, TPB * 1024], bf16, tag="ptc")
                for i in range(TPB):
                    for qh in range(2):
                        stp = psa.tile([128, 512], f32, tag="psa")
                        for ck in range(CK):
                            nc.tensor.matmul(
                                stp[:],
                                xtt[:, ck * TB + i * 128: ck * TB + (i + 1) * 128],
                                gt[:, ck * 1024 + qh * 512: ck * 1024 + (qh + 1) * 512],
                                start=(ck == 0), stop=(ck == CK - 1))
                        pslice = ptc[:, i * 1024 + qh * 512: i * 1024 + (qh + 1) * 512]
                        nc.scalar.activation(pslice, stp[:], EXP, scale=SCALE)
                        pa = pacc[:, qh * 512:(qh + 1) * 512]
                        nc.gpsimd.tensor_add(pa, pslice, pa)

                # u^T accumulation over block pairs, N=512
                if blk % 2 == 0:
                    s["ptc_prev"], s["xnt_prev"] = ptc, xnt
                    return
                ut, un = s["ut"], s["un"]
                last = blk == NBLK - 1
                tail = last and interleave_tail
                for rnd, (qh, cq) in enumerate(
                        [(q, c) for q in range(2) for c in range(2)]):
                    ups = [psu.tile([128, 512], f32, tag="ups",
                                    name=f"ups{b}_{blk}_{qh}_{cq}_{j}")
                           for j in range(4)]
                    # in the very last round, finish each k4's accumulation
                    # before starting the next so its eviction (and the o
                    # matmuls contracting it) unblock ~5us earlier
                    k4_serial = tail and rnd == 3
                    if k4_serial:
                        order = [(k4, half, i) for k4 in range(4)
                                 for half in range(2) for i in range(TPB)]
                    else:
                        order = [(k4, half, i) for half in range(2)
                                 for i in range(TPB) for k4 in range(4)]
                    for k4, half, i in order:
                        pp, xx = ((s["ptc_prev"], s["xnt_prev"]),
                                  (ptc, xnt))[half]
                        ck = cq * 4 + k4
                        nc.tensor.matmul(
                            ups[k4][:],
                            xx[:, i * 1024 + ck * 128: i * 1024 + (ck + 1) * 128],
                            pp[:, i * 1024 + qh * 512: i * 1024 + (qh + 1) * 512],
                            start=(half == 0 and i == 0),
                            stop=(half == 1 and i == TPB - 1))
                    if k4_serial:
                        for k4 in range(4):
                            ck = cq * 4 + k4
                            nc.vector.tensor_add(
                                un[:, ck * 1024 + qh * 512: ck * 1024 + (qh + 1) * 512],
                                ups[k4][:],
                                ut[:, ck * 1024 + qh * 512: ck * 1024 + (qh + 1) * 512])
                        continue
                    for k4 in range(4):
                        ck = cq * 4 + k4
                        dst = ut[:, ck * 1024 + qh * 512: ck * 1024 + (qh + 1) * 512]
                        if blk == 1:
                            nc.vector.tensor_copy(dst, ups[k4][:])
                        elif last:
                            nc.vector.tensor_add(
                                un[:, ck * 1024 + qh * 512: ck * 1024 + (qh + 1) * 512],
                                ups[k4][:], dst)
                        else:
                            nc.vector.tensor_add(dst, ups[k4][:], dst)
                    if tail and rnd == 0:
                        emit_den(b)      # pacc long complete; runs on ACT/DVE
                    if tail and rnd == 2:
                        # o first half for heads 0-7: (qh0,cq0) columns of un
                        # are evicted by now — overlap with the last u round
                        emit_o(b, range(4), 0, 4, start=True, stop=False)
                if tail:
                    emit_o(b, range(4), 4, 8, start=False, stop=True)
                    emit_o(b, range(4, 8), 0, 8, start=True, stop=True)

            def emit_den(b):
                pacc = paccs[b]
                # den replicated on all 128 partitions via all-ones lhsT
                # (bf16 rounding of per-partition partials: ~0.02% on den);
                # fast-approx reciprocal.
                paccb = small.tile([128, HQ], bf16, tag="paccb",
                                   name=f"paccb{b}")
                rdf = small.tile([128, HQ], f32, tag="rdf", name=f"rdf{b}")
                for qh in range(2):
                    nc.scalar.activation(paccb[:, qh * 512:(qh + 1) * 512],
                                         pacc[:, qh * 512:(qh + 1) * 512], CPY)
                    dnp = psu.tile([128, 512], f32, tag="ups",
                                   name=f"dnp{b}_{qh}")
                    nc.tensor.matmul(dnp[:], ones128[:],
                                     paccb[:, qh * 512:(qh + 1) * 512],
                                     start=True, stop=True)
                    nc.vector.reciprocal_approx_fast(
                        rdf[:, qh * 512:(qh + 1) * 512], dnp[:])
                # per-head-pair reciprocal layout for the oT scale:
                # rdo[p, jc*64+qq] = 1/d[(2jc + p//64)*64 + qq]
                # (two strided copies on the scalar engine)
                rdo = small.tile([128, 8 * QL], f32, tag="rdo", name=f"rdo{b}")
                for half in range(2):
                    src = rdf[half * 64:(half + 1) * 64, :].rearrange(
                        "p (j t q) -> p j t q", j=8, t=2)[:, :, half, :]
                    dst = rdo[half * 64:(half + 1) * 64, :].rearrange(
                        "p (j q) -> p j q", j=8)
                    nc.scalar.activation(dst, src, CPY)
                st[b]["rdo"] = rdo
                st[b]["oT"] = small.tile([128, 8 * QL], bf16, tag="oT",
                                         name=f"oT{b}")

            def emit_o(b, jcs, ck_lo, ck_hi, start, stop):
                un = st[b]["un"]
                rdo, oT = st[b]["rdo"], st[b]["oT"]
                ops = st[b].setdefault("ops", {})
                for jc in jcs:  # head pair (2jc, 2jc+1)
                    if start:
                        ops[jc] = psa.tile([128, 512], f32, tag="psa",
                                           name=f"o{b}_{jc}")
                    ps = ops[jc]
                    for sub in range(2):
                        h = jc * 2 + sub
                        ucol = (h // 8) * 512 + (h % 8) * 64
                        for ck in range(ck_lo, ck_hi):
                            nc.tensor.matmul(
                                ps[sub * 64:(sub + 1) * 64, 0:QL],
                                wt3[:, ck * 1024 + h * 64: ck * 1024 + (h + 1) * 64],
                                un[:, ck * 1024 + ucol: ck * 1024 + ucol + 64],
                                start=(start and ck == ck_lo),
                                stop=(stop and ck == ck_hi - 1),
                                tile_position=(0, sub * 64))
                    if stop:
                        nc.vector.tensor_mul(oT[:, jc * QL:(jc + 1) * QL],
                                             ps[:, 0:QL],
                                             rdo[:, jc * QL:(jc + 1) * QL])

            def emit_tail(b):
                oT = st[b]["oT"]
                ys = small.tile([128, C], f32, tag="ys", name=f"ys{b}")
                for half in range(2):
                    ps = psa.tile([128, 512], f32, tag="psa")
                    for jc in range(8):
                        nc.tensor.matmul(
                            ps[0:QL, :],
                            oT[:, jc * QL:(jc + 1) * QL],
                            wt4[:, jc * 1024 + half * 512: jc * 1024 + (half + 1) * 512],
                            start=(jc == 0), stop=(jc == 7))
                    nc.vector.tensor_add(
                        ys[0:QL, half * 512:(half + 1) * 512], ps[0:QL, :],
                        bpf[0:QL, half * 512:(half + 1) * 512])
                    # flush each half as soon as its bias add lands so the
                    # first 256KB store overlaps the second proj half
                    nc.sync.dma_start(y[b, :, half * 512:(half + 1) * 512],
                                      ys[0:QL, half * 512:(half + 1) * 512])

            def emit_epilogue(b):
                emit_den(b)
                emit_o(b, range(8), 0, 8, start=True, stop=True)
                emit_tail(b)

            # ---------- main emission: interleave batch boundary ----------
            emit_block(0, 0, pre=(xtt0, xnt0))
            emit_block(0, 1)
            emit_block(0, 2)
            # epilogue weights stream during the t-loop (reuse wpool slots);
            # deferred so they don't steal HBM bandwidth from the startup path
            wt3 = wpool.tile([128, 8 * 1024], bf16, tag="w", name="wt_v")
            nc.sync.dma_start(
                wt3[:].rearrange("p (t c) -> p t c", t=CK),
                wv[:, :].rearrange("(t p) c -> p t c", p=128))
            wt4 = wpool.tile([128, 8 * 1024], bf16, tag="w", name="wt_p")
            nc.sync.dma_start(
                wt4[:].rearrange("p (t c) -> p t c", t=CK),
                wp[:, :].rearrange("(t p) c -> p t c", p=128))
            for blk in range(3, NBLK):
                emit_block(0, blk)
            emit_block(1, 0)
            emit_epilogue(0)      # hidden under b1 block 0/1 matmuls
            for blk in range(1, NBLK - 1):
                emit_block(1, blk)
            # last block: den chain + o matmuls interleaved into the u rounds
            emit_block(1, NBLK - 1, interleave_tail=True)
            emit_tail(1)

    nc.compile()
    return nc


def get_nc():
    if "nc" not in _CACHE:
        _CACHE["nc"] = _build()
    return _CACHE["nc"]


def make_in_maps(x, Wq, Wk, Wv, Wproj, bproj):
    import ml_dtypes
    bf = ml_dtypes.bfloat16
    x = np.ascontiguousarray(x, dtype=np.float32)
    xt32 = np.ascontiguousarray(x.transpose(0, 2, 1))
    xtb = xt32.astype(bf)
    xnb = x.astype(bf)
    wqb = np.ascontiguousarray(np.asarray(Wq, dtype=np.float32).T).astype(bf)
    # wk2[p, ck*1024 + pair*128 + m] = Wk[pair*128 + p, ck*128 + m]
    wkb = np.ascontiguousarray(
        np.asarray(Wk, dtype=np.float32).reshape(8, 128, 8, 128)
        .transpose(1, 2, 0, 3).reshape(128, 8 * 1024)).astype(bf)
    wvb = np.ascontiguousarray(np.asarray(Wv, dtype=np.float32).T).astype(bf)
    wpb = np.ascontiguousarray(np.asarray(Wproj, dtype=np.float32).T).astype(bf)
    bpf = np.ascontiguousarray(
        np.asarray(bproj, dtype=np.float32).reshape(1, C)).astype(bf)
    in_maps = []
    for core in range(8):
        s = slice(core * BL, (core + 1) * BL)
        # xq2[c, b*64+q] = x^T[b, c, q] for the core's two batches
        xq2 = np.concatenate([xt32[core * BL + b, :, 0:QL] for b in range(BL)],
                             axis=1).astype(bf)
        in_maps.append({
            "xn": np.ascontiguousarray(xnb[s]),
            "xt": np.ascontiguousarray(xtb[s]),
            "xq2": np.ascontiguousarray(xq2),
            "wq": wqb, "wk2": wkb, "wv": wvb, "wp": wpb, "bp": bpf,
        })
    return in_maps


def kernel(x, Wq, Wk, Wv, Wproj, bproj):
    from concourse import bass_utils
    nc = get_nc()
    in_maps = make_in_maps(x, Wq, Wk, Wv, Wproj, bproj)
    res = bass_utils.run_bass_kernel_spmd(nc, in_maps, core_ids=list(range(8)))
    out = np.concatenate([res.results[i]["y"] for i in range(8)], axis=0)
    return out.astype(np.float32)


# revision 33
# speedup vs baseline: 1.0100x; 1.0073x over previous
"""Trainium2 Bass kernel for nn_Attention (B=16,N=4096,C=1024,H=16,HD=64,Q=64).

Data-parallel over B across 8 NeuronCores (2 batches/core). Per batch the
attention is reassociated so no k/v tensors are materialized and no on-chip
transposes are needed:

  q^T = Wq @ x_q^T                      [(h,d)=1024, 64]
  G_h^T = Wk_h^T @ q_h                  G^T: [c=1024, (h,q)=1024]
  S^T   = x @ G^T   (per t-tile)        [t, (h,q)]   (contract c)
  p^T   = exp(S^T / 8)                  (softmax w/o max-sub: scores ~ +-5)
  u^T   = x^T(nat) @ p^T  (accum t)     [c, (h,q)]   (contract t)
  den   = ones @ pacc     (pacc: GpSimd p-sum over t)
  o_h^T = (Wv_h^T)^T @ u_h^T, scaled by 1/den at PSUM eviction
  y     = o^T.T @ Wproj^T + b           [64, 1024]   (contract (h,d))

Optimizations over the first working version:
  - startup: consolidated/prioritized DMAs (order xq2 -> wq -> x-block0 ->
    wk -> xnt0; wv/wp deferred past block 2) with per-ck granularity so the
    q/G matmuls pipeline with the weight stream; wk pre-tiled on host so its
    DMA runs full-bandwidth 2KB lines.
  - q matmuls packed across both batches (N=128), G matmuls packed (N=256),
    PSUM evictions split across DVE and ACT.
  - batch 1's block 0 is emitted before batch 0's epilogue so the PE never
    drains at the batch boundary (keeps HAM at K=8/8).
  - softmax denominators: bf16 pacc copy -> all-ones matmul replicates den
    across partitions (no GpSimd broadcast), reciprocal_approx_fast, rdo via
    two strided ACT copies; dn PSUM tiles live in the u pool so o matmuls
    keep all 4 psa banks.
  - last batch's tail: den chain + o matmuls interleaved into the final u
    rounds (o first-half runs while the last u round computes), so only the
    last 4 PSUM evictions + o second-half + proj remain after the last u
    matmul.
"""
import os
import numpy as np

B, N, C = 16, 4096, 1024
H, HD, QL = 16, 64, 64
BL = B // 8           # batches per core
CK = C // 128         # 8 c-tiles
TB = 512              # tokens per t-block
NBLK = N // TB        # 8 blocks
TPB = TB // 128       # 4 t-tiles per block
HQ = H * QL           # 1024
SCALE = HD ** -0.5

_CACHE = {}


def _build():
    import concourse.bass as bass
    import concourse.tile as tile
    from concourse import bacc, mybir

    f32 = mybir.dt.float32
    bf16 = mybir.dt.bfloat16
    EXP = mybir.ActivationFunctionType.Exp
    CPY = mybir.ActivationFunctionType.Copy

    nc = bacc.Bacc("TRN2", target_bir_lowering=False, debug=False, num_devices=8)
    xn = nc.dram_tensor("xn", [BL, N, C], bf16, kind="ExternalInput").ap()
    xt = nc.dram_tensor("xt", [BL, C, N], bf16, kind="ExternalInput").ap()
    wq = nc.dram_tensor("wq", [C, C], bf16, kind="ExternalInput").ap()   # Wq^T
    # Wk pre-tiled on host: wk2[p, ck*1024+pair*128+m] = Wk[pair*128+p, ck*128+m]
    wk2 = nc.dram_tensor("wk2", [128, CK * 1024], bf16, kind="ExternalInput").ap()
    wv = nc.dram_tensor("wv", [C, C], bf16, kind="ExternalInput").ap()   # Wv^T
    wp = nc.dram_tensor("wp", [C, C], bf16, kind="ExternalInput").ap()   # Wproj^T
    bp = nc.dram_tensor("bp", [1, C], bf16, kind="ExternalInput").ap()
    # xq2[c, b*64+q] = x[b, q, c] for this core's two batches
    xq2 = nc.dram_tensor("xq2", [C, 2 * QL], bf16, kind="ExternalInput").ap()
    y = nc.dram_tensor("y", [BL, QL, C], f32, kind="ExternalOutput").ap()

    with tile.TileContext(nc) as tc:
        with (
            tc.tile_pool(name="wpool", bufs=2) as wpool,
            tc.tile_pool(name="xpool", bufs=2) as xpool,
            tc.tile_pool(name="gpool", bufs=1) as gpool,
            tc.tile_pool(name="upool", bufs=1) as upool,
            tc.tile_pool(name="small", bufs=1) as small,
            tc.tile_pool(name="ptp", bufs=3) as ptp,
            tc.tile_pool(name="psa", bufs=4, space="PSUM") as psa,
            tc.tile_pool(name="psu", bufs=4, space="PSUM") as psu,
        ):
            # ---------- memsets first: no DMA deps, run under the launch ----
            ones128 = small.tile([128, 128], bf16, tag="ones128")
            nc.gpsimd.memset(ones128[:], 1.0)
            qbd2 = small.tile([128, 8 * 256], bf16, tag="qbd2")
            nc.gpsimd.memset(qbd2[:], 0.0)
            paccs = []
            for b in range(BL):
                pacc = small.tile([128, HQ], f32, tag="pacc", bufs=2,
                                  name=f"pacc{b}")
                nc.gpsimd.memset(pacc[:], 0.0)
                paccs.append(pacc)

            # ---------- startup DMAs in dependency-priority order ----------
            xqt2 = small.tile([128, CK * 128], bf16, tag="xqt")
            nc.sync.dma_start(
                xqt2[:].rearrange("p (t q) -> p t q", t=CK),
                xq2[:, :].rearrange("(t p) q -> p t q", p=128))
            # Wq^T per c-tile so q matmuls pipeline with the stream
            wt = wpool.tile([128, 8 * 1024], bf16, tag="w", name="wt_q")
            for ck in range(CK):
                nc.sync.dma_start(wt[:, ck * 1024:(ck + 1) * 1024],
                                  wq[ck * 128:(ck + 1) * 128, :])
            # block-0 x^T ahead of wk: the first four S tiles are fused
            # into the G loop below (on the psu ring) and need it early
            xtt0 = xpool.tile([128, CK * TB], bf16, tag="xt", bufs=2)
            nc.sync.dma_start(
                xtt0[:].rearrange("p (t n) -> p t n", t=CK),
                xt[0, :, 0:TB].rearrange("(t p) n -> p t n", p=128))
            # Wk host-pre-tiled column-tile-major: wt2[p, ck*1024+pair*128+m]
            #   = wk[pair*128 + p, ck*128 + m]; G's ck-tiles unblock in order.
            wt2 = wpool.tile([128, 8 * 1024], bf16, tag="w", name="wt_k")
            for ck in range(CK):
                nc.sync.dma_start(wt2[:, ck * 1024:(ck + 1) * 1024],
                                    wk2[:, ck * 1024:(ck + 1) * 1024])
            xnt0 = xpool.tile([128, TPB * 1024], bf16, tag="xn", bufs=3)
            nc.sync.dma_start(
                xnt0[:].rearrange("p (t c) -> p t c", t=TPB),
                xn[0, 0:TB, :].rearrange("(t p) c -> p t c", p=128))
            bps = small.tile([128, C], bf16, tag="bps")
            nc.sync.dma_start(bps[0:1, :], bp[:, :])
            bpf = small.tile([128, C], bf16, tag="bpf")
            nc.gpsimd.partition_broadcast(bpf[:], bps[0:1, :])

            # ---------- PE warm-up: ~12 junk matmuls under the DMA wait ----
            # (HAM releases the 4/8 clock throttle after ~3.4us of sustained
            # PE activity; this burst runs while wq streams so the real
            # q/G/S matmuls start at 2.4GHz)
            psw = psa.tile([128, 512], f32, tag="psa", name="warm")
            for w in range(12):
                nc.tensor.matmul(psw[:], qbd2[:, 0:128], qbd2[:, 0:512],
                                 start=(w == 0), stop=(w == 11))

            # ---------- q matmuls, both batches packed (N=128) ----------
            # out chunk jc rows = (h,d) of heads (2jc, 2jc+1); cols b*64+q.
            # Evictions build the block-diagonal qbd2: per pair slot of 256
            # cols, [b0 blockdiag 128 | b1 blockdiag 128].
            for jg in range(2):
                ps = psa.tile([128, 512], f32, tag="psa")
                for sub in range(4):
                    jc = jg * 4 + sub
                    for ck in range(CK):
                        nc.tensor.matmul(
                            ps[:, sub * 128:(sub + 1) * 128],
                            wt[:, ck * 1024 + jc * 128: ck * 1024 + (jc + 1) * 128],
                            xqt2[:, ck * 128:(ck + 1) * 128],
                            start=(ck == 0), stop=(ck == CK - 1))
                for sub in range(4):
                    jc = jg * 4 + sub
                    for b in range(BL):
                        base = jc * 256 + b * 128
                        src0 = ps[0:64, sub * 128 + b * 64: sub * 128 + b * 64 + 64]
                        src1 = ps[64:128, sub * 128 + b * 64: sub * 128 + b * 64 + 64]
                        eng = nc.vector if (sub + b) % 2 == 0 else nc.scalar
                        if eng is nc.vector:
                            nc.vector.tensor_copy(qbd2[0:64, base:base + 64], src0)
                            nc.vector.tensor_copy(qbd2[64:128, base + 64:base + 128], src1)
                        else:
                            nc.scalar.activation(qbd2[0:64, base:base + 64], src0, CPY)
                            nc.scalar.activation(qbd2[64:128, base + 64:base + 128], src1, CPY)

            # ---------- G matmuls, both batches packed (N=256) ----------
            gts = []
            for b in range(BL):
                gts.append(gpool.tile([128, CK * 1024], bf16, tag="gt", bufs=2,
                                      name=f"gt{b}"))
            # block-0 ptc + the first four S tiles accumulate on the psu
            # ring while G streams: their matmuls fill the PE gaps where G
            # waits on the wk DMA (each needs only gt c-tiles already
            # evicted)
            ptc0 = ptp.tile([128, TPB * 1024], bf16, tag="ptc")
            sfuse = [(0, 0), (0, 1), (1, 0), (1, 1)]
            spsu = {iq: psu.tile([128, 512], f32, tag="ups",
                                 name=f"sfuse{iq[0]}_{iq[1]}")
                    for iq in sfuse}
            for ck in range(CK):
                for pg in range(4):
                    ps = psa.tile([128, 512], f32, tag="psa")
                    for s2 in range(2):
                        pair = pg * 2 + s2
                        nc.tensor.matmul(
                            ps[:, s2 * 256:(s2 + 1) * 256],
                            wt2[:, ck * 1024 + pair * 128: ck * 1024 + (pair + 1) * 128],
                            qbd2[:, pair * 256:(pair + 1) * 256],
                            start=True, stop=True)
                    ps4 = ps[:].rearrange("p (t b n) -> p t b n", t=2, b=2, n=128)
                    for b in range(BL):
                        dst = gts[b][:, ck * 1024 + pg * 256: ck * 1024 + (pg + 1) * 256]
                        dst3 = dst.rearrange("p (t n) -> p t n", t=2)
                        if (pg + b) % 2 == 0:
                            nc.vector.tensor_copy(dst3, ps4[:, :, b, :])
                        else:
                            nc.scalar.activation(dst3, ps4[:, :, b, :], CPY)
                for i, qh in sfuse:
                    nc.tensor.matmul(
                        spsu[(i, qh)][:],
                        xtt0[:, ck * TB + i * 128: ck * TB + (i + 1) * 128],
                        gts[0][:, ck * 1024 + qh * 512: ck * 1024 + (qh + 1) * 512],
                        start=(ck == 0), stop=(ck == CK - 1))

            for i, qh in sfuse:
                pslice = ptc0[:, i * 1024 + qh * 512: i * 1024 + (qh + 1) * 512]
                nc.scalar.activation(pslice, spsu[(i, qh)][:], EXP, scale=SCALE)
                pa = paccs[0][:, qh * 512:(qh + 1) * 512]
                nc.gpsimd.tensor_add(pa, pslice, pa)

            # ---------- t-loop / epilogue emission helpers ----------
            st = [dict(ptc_prev=None, xnt_prev=None, ut=None, un=None)
                  for _ in range(BL)]

            def emit_block(b, blk, interleave_tail=False, pre=None,
                           ptc_pre=None, skip=0):
                s = st[b]
                gt = gts[b]
                pacc = paccs[b]
                if blk == 1:
                    s["ut"] = upool.tile([128, CK * 1024], f32, tag="ut",
                                         name=f"ut{b}")
                    s["un"] = gpool.tile([128, CK * 1024], bf16, tag="un",
                                         name=f"un{b}")
                if pre is not None:
                    xtt, xnt = pre
                else:
                    xtt = xpool.tile([128, CK * TB], bf16, tag="xt", bufs=2)
                    nc.sync.dma_start(
                        xtt[:].rearrange("p (t n) -> p t n", t=CK),
                        xt[b, :, blk * TB:(blk + 1) * TB].rearrange(
                            "(t p) n -> p t n", p=128))
                    xnt = xpool.tile([128, TPB * 1024], bf16, tag="xn", bufs=3)
                    nc.sync.dma_start(
                        xnt[:].rearrange("p (t c) -> p t c", t=TPB),
                        xn[b, blk * TB:(blk + 1) * TB, :].rearrange(
                            "(t p) c -> p t c", p=128))

                # S^T + exp into p cache; pacc accumulates p on GpSimd
                ptc = ptc_pre if ptc_pre is not None else ptp.tile(
                    [128, TPB * 1024], bf16, tag="ptc")
                for i in range(TPB):
                    for qh in range(2):
                        if i * 2 + qh < skip:
                            continue
                        stp = psa.tile([128, 512], f32, tag="psa")
                        for ck in range(CK):
                            nc.tensor.matmul(
                                stp[:],
                                xtt[:, ck * TB + i * 128: ck * TB + (i + 1) * 128],
                                gt[:, ck * 1024 + qh * 512: ck * 1024 + (qh + 1) * 512],
                                start=(ck == 0), stop=(ck == CK - 1))
                        pslice = ptc[:, i * 1024 + qh * 512: i * 1024 + (qh + 1) * 512]
                        nc.scalar.activation(pslice, stp[:], EXP, scale=SCALE)
                        pa = pacc[:, qh * 512:(qh + 1) * 512]
                        nc.gpsimd.tensor_add(pa, pslice, pa)

                # u^T accumulation over block pairs, N=512
                if blk % 2 == 0:
                    s["ptc_prev"], s["xnt_prev"] = ptc, xnt
                    return
                ut, un = s["ut"], s["un"]
                last = blk == NBLK - 1
                tail = last and interleave_tail
                for rnd, (qh, cq) in enumerate(
                        [(q, c) for q in range(2) for c in range(2)]):
                    ups = [psu.tile([128, 512], f32, tag="ups",
                                    name=f"ups{b}_{blk}_{qh}_{cq}_{j}")
                           for j in range(4)]
                    # in the very last round, finish each k4's accumulation
                    # before starting the next so its eviction (and the o
                    # matmuls contracting it) unblock ~5us earlier
                    k4_serial = tail and rnd == 3
                    if k4_serial:
                        order = [(k4, half, i) for k4 in range(4)
                                 for half in range(2) for i in range(TPB)]
                    else:
                        order = [(k4, half, i) for half in range(2)
                                 for i in range(TPB) for k4 in range(4)]
                    for k4, half, i in order:
                        pp, xx = ((s["ptc_prev"], s["xnt_prev"]),
                                  (ptc, xnt))[half]
                        ck = cq * 4 + k4
                        nc.tensor.matmul(
                            ups[k4][:],
                            xx[:, i * 1024 + ck * 128: i * 1024 + (ck + 1) * 128],
                            pp[:, i * 1024 + qh * 512: i * 1024 + (qh + 1) * 512],
                            start=(half == 0 and i == 0),
                            stop=(half == 1 and i == TPB - 1))
                    if k4_serial:
                        for k4 in range(4):
                            ck = cq * 4 + k4
                            nc.vector.tensor_add(
                                un[:, ck * 1024 + qh * 512: ck * 1024 + (qh + 1) * 512],
                                ups[k4][:],
                                ut[:, ck * 1024 + qh * 512: ck * 1024 + (qh + 1) * 512])
                        continue
                    for k4 in range(4):
                        ck = cq * 4 + k4
                        dst = ut[:, ck * 1024 + qh * 512: ck * 1024 + (qh + 1) * 512]
                        if blk == 1:
                            nc.vector.tensor_copy(dst, ups[k4][:])
                        elif last:
                            nc.vector.tensor_add(
                                un[:, ck * 1024 + qh * 512: ck * 1024 + (qh + 1) * 512],
                                ups[k4][:], dst)
                        else:
                            nc.vector.tensor_add(dst, ups[k4][:], dst)
                    if tail and rnd == 0:
                        emit_den(b)      # pacc long complete; runs on ACT/DVE
                    if tail and rnd == 2:
                        # o first half for heads 0-7: (qh0,cq0) columns of un
                        # are evicted by now — overlap with the last u round
                        emit_o(b, range(4), 0, 4, start=True, stop=False)
                if tail:
                    emit_o(b, range(4), 4, 8, start=False, stop=True)
                    emit_o(b, range(4, 8), 0, 8, start=True, stop=True)

            def emit_den(b):
                pacc = paccs[b]
                # den replicated on all 128 partitions via all-ones lhsT
                # (bf16 rounding of per-partition partials: ~0.02% on den);
                # fast-approx reciprocal.
                paccb = small.tile([128, HQ], bf16, tag="paccb",
                                   name=f"paccb{b}")
                rdf = small.tile([128, HQ], f32, tag="rdf", name=f"rdf{b}")
                for qh in range(2):
                    nc.scalar.activation(paccb[:, qh * 512:(qh + 1) * 512],
                                         pacc[:, qh * 512:(qh + 1) * 512], CPY)
                    dnp = psu.tile([128, 512], f32, tag="ups",
                                   name=f"dnp{b}_{qh}")
                    nc.tensor.matmul(dnp[:], ones128[:],
                                     paccb[:, qh * 512:(qh + 1) * 512],
                                     start=True, stop=True)
                    nc.vector.reciprocal_approx_fast(
                        rdf[:, qh * 512:(qh + 1) * 512], dnp[:])
                # per-head-pair reciprocal layout for the oT scale:
                # rdo[p, jc*64+qq] = 1/d[(2jc + p//64)*64 + qq]
                # (two strided copies on the scalar engine)
                rdo = small.tile([128, 8 * QL], f32, tag="rdo", name=f"rdo{b}")
                for half in range(2):
                    src = rdf[half * 64:(half + 1) * 64, :].rearrange(
                        "p (j t q) -> p j t q", j=8, t=2)[:, :, half, :]
                    dst = rdo[half * 64:(half + 1) * 64, :].rearrange(
                        "p (j q) -> p j q", j=8)
                    nc.scalar.activation(dst, src, CPY)
                st[b]["rdo"] = rdo
                st[b]["oT"] = small.tile([128, 8 * QL], bf16, tag="oT",
                                         name=f"oT{b}")

            def emit_o(b, jcs, ck_lo, ck_hi, start, stop):
                un = st[b]["un"]
                rdo, oT = st[b]["rdo"], st[b]["oT"]
                ops = st[b].setdefault("ops", {})
                for jc in jcs:  # head pair (2jc, 2jc+1)
                    if start:
                        ops[jc] = psa.tile([128, 512], f32, tag="psa",
                                           name=f"o{b}_{jc}")
                    ps = ops[jc]
                    for sub in range(2):
                        h = jc * 2 + sub
                        ucol = (h // 8) * 512 + (h % 8) * 64
                        for ck in range(ck_lo, ck_hi):
                            nc.tensor.matmul(
                                ps[sub * 64:(sub + 1) * 64, 0:QL],
                                wt3[:, ck * 1024 + h * 64: ck * 1024 + (h + 1) * 64],
                                un[:, ck * 1024 + ucol: ck * 1024 + ucol + 64],
                                start=(start and ck == ck_lo),
                                stop=(stop and ck == ck_hi - 1),
                                tile_position=(0, sub * 64))
                    if stop:
                        nc.vector.tensor_mul(oT[:, jc * QL:(jc + 1) * QL],
                                             ps[:, 0:QL],
                                             rdo[:, jc * QL:(jc + 1) * QL])

            def emit_tail(b):
                oT = st[b]["oT"]
                ys = small.tile([128, C], f32, tag="ys", name=f"ys{b}")
                for half in range(2):
                    ps = psa.tile([128, 512], f32, tag="psa")
                    for jc in range(8):
                        nc.tensor.matmul(
                            ps[0:QL, :],
                            oT[:, jc * QL:(jc + 1) * QL],
                            wt4[:, jc * 1024 + half * 512: jc * 1024 + (half + 1) * 512],
                            start=(jc == 0), stop=(jc == 7))
                    nc.vector.tensor_add(
                        ys[0:QL, half * 512:(half + 1) * 512], ps[0:QL, :],
                        bpf[0:QL, half * 512:(half + 1) * 512])
                    # flush each half as soon as its bias add lands so the
                    # first 256KB store overlaps the second proj half
                    nc.sync.dma_start(y[b, :, half * 512:(half + 1) * 512],
                                      ys[0:QL, half * 512:(half + 1) * 512])

            def emit_epilogue(b):
                emit_den(b)
                emit_o(b, range(8), 0, 8, start=True, stop=True)
                emit_tail(b)

            # ---------- main emission: interleave batch boundary ----------
            emit_block(0, 0, pre=(xtt0, xnt0), ptc_pre=ptc0, skip=4)
            emit_block(0, 1)
            emit_block(0, 2)
            # epilogue weights stream during the t-loop (reuse wpool slots);
            # deferred so they don't steal HBM bandwidth from the startup path
            wt3 = wpool.tile([128, 8 * 1024], bf16, tag="w", name="wt_v")
            nc.sync.dma_start(
                wt3[:].rearrange("p (t c) -> p t c", t=CK),
                wv[:, :].rearrange("(t p) c -> p t c", p=128))
            wt4 = wpool.tile([128, 8 * 1024], bf16, tag="w", name="wt_p")
            nc.sync.dma_start(
                wt4[:].rearrange("p (t c) -> p t c", t=CK),
                wp[:, :].rearrange("(t p) c -> p t c", p=128))
            for blk in range(3, NBLK):
                emit_block(0, blk)
            emit_block(1, 0)
            emit_epilogue(0)      # hidden under b1 block 0/1 matmuls
            for blk in range(1, NBLK - 1):
                emit_block(1, blk)
            # last block: den chain + o matmuls interleaved into the u rounds
            emit_block(1, NBLK - 1, interleave_tail=True)
            emit_tail(1)

    nc.compile()
    return nc


def get_nc():
    if "nc" not in _CACHE:
        _CACHE["nc"] = _build()
    return _CACHE["nc"]


def make_in_maps(x, Wq, Wk, Wv, Wproj, bproj):
    import ml_dtypes
    bf = ml_dtypes.bfloat16
    x = np.ascontiguousarray(x, dtype=np.float32)
    xt32 = np.ascontiguousarray(x.transpose(0, 2, 1))
    xtb = xt32.astype(bf)
    xnb = x.astype(bf)
    wqb = np.ascontiguousarray(np.asarray(Wq, dtype=np.float32).T).astype(bf)
    # wk2[p, ck*1024 + pair*128 + m] = Wk[pair*128 + p, ck*128 + m]
    wkb = np.ascontiguousarray(
        np.asarray(Wk, dtype=np.float32).reshape(8, 128, 8, 128)
        .transpose(1, 2, 0, 3).reshape(128, 8 * 1024)).astype(bf)
    wvb = np.ascontiguousarray(np.asarray(Wv, dtype=np.float32).T).astype(bf)
    wpb = np.ascontiguousarray(np.asarray(Wproj, dtype=np.float32).T).astype(bf)
    bpf = np.ascontiguousarray(
        np.asarray(bproj, dtype=np.float32).reshape(1, C)).astype(bf)
    in_maps = []
    for core in range(8):
        s = slice(core * BL, (core + 1) * BL)
        # xq2[c, b*64+q] = x^T[b, c, q] for the core's two batches
        xq2 = np.concatenate([xt32[core * BL + b, :, 0:QL] for b in range(BL)],
                             axis=1).astype(bf)
        in_maps.append({
            "xn": np.ascontiguousarray(xnb[s]),
            "xt": np.ascontiguousarray(xtb[s]),
            "xq2": np.ascontiguousarray(xq2),
            "wq": wqb, "wk2": wkb, "wv": wvb, "wp": wpb, "bp": bpf,
        })
    return in_maps


def kernel(x, Wq, Wk, Wv, Wproj, bproj):
    from concourse import bass_utils
    nc = get_nc()
    in_maps = make_in_maps(x, Wq, Wk, Wv, Wproj, bproj)
    res = bass_utils.run_bass_kernel_spmd(nc, in_maps, core_ids=list(range(8)))
    out = np.concatenate([res.results[i]["y"] for i in range(8)], axis=0)
    return out.astype(np.float32)
